# revision 1
# baseline (speedup 1.0000x reference)
"""Causal self-attention (B=4, S=2048, C=1024, 16 heads) on 8 Trainium2 cores.

Sharding: 8 cores = 4 batches x 2 head-groups (8 heads each).
Each core computes, for its (batch b, head-group g):
  qkT = (Wqk_g.T @ x_b.T) + bqk_g          [1024, 2048]  (q rows then k rows)
  v   = (x_b @ Wv_g) + bv_g                [2048, 520]   (65-strided heads, ones col)
  per head h, q-chunk qc (512 wide), k-chunk kc (128 wide, causal band only):
    scoresT = kT_chunk.T @ qT_chunk        [128, <=512]  (K = head dim 64)
    expT    = exp(0.125 * scoresT); triangular mask on the diagonal subtile
    yaccT  += [v_chunk | ones].T @ expT    [65, <=512]   (row 64 = softmax denom)
  yT = yaccT[0:64] * broadcast(1/yaccT[64])               (broadcast via K=1 matmul)
  outT_partial = Wproj_rows_g.T @ yT + bproj (g==0 only)  [1024, 2048]
Host sums the two head-group partials per batch and transposes.

Matmuls run as float32r (TF32, full PE rate at free-dim >=256). Score matmuls
for even/odd head pairs are row-packed into PE array halves via tile_position.
Diagonal-band blocks are column-trimmed to the causal region.
"""
import sys

sys.path.insert(0, "/opt/trn_rl_repo")

import numpy as np

S = 2048
C = 1024
NH = 16
NE = 64
G = 2                 # head groups (tensor-parallel factor)
HG = NH // G          # heads per core = 8
SQ = 512              # q chunk
SK = 128              # k chunk
KC = C // 128         # 8 contraction chunks for qkv projections
NQ = S // SQ
N_CORES = 8

_BUILT = None


def _build():
    global _BUILT
    if _BUILT is not None:
        return _BUILT

    import concourse.bacc as bacc
    import concourse.mybir as mybir
    from concourse import tile

    dt = mybir.dt
    F32 = dt.float32
    F32R = dt.float32r
    AF = mybir.ActivationFunctionType
    BF16 = dt.bfloat16

    nc = bacc.Bacc("TRN2", target_bir_lowering=False, debug=False, num_devices=N_CORES)

    xT_d = nc.dram_tensor("xp", [128, KC * S], BF16, kind="ExternalInput").ap()
    wqk_d = nc.dram_tensor("wqkp", [128, KC * 1024], BF16, kind="ExternalInput").ap()
    wv_d = nc.dram_tensor("wvp", [128, KC * 512], BF16, kind="ExternalInput").ap()
    bqk_d = nc.dram_tensor("bqk", [128, 8], F32, kind="ExternalInput").ap()
    bv_d = nc.dram_tensor("bv", [1, 512], F32, kind="ExternalInput").ap()
    wp_d = nc.dram_tensor("wpp", [128, 4 * C], F32R, kind="ExternalInput").ap()
    bp_d = nc.dram_tensor("bp", [128, 8], F32, kind="ExternalInput").ap()
    tri_d = nc.dram_tensor("tri", [128, 128], F32, kind="ExternalInput").ap()
    onesc_d = nc.dram_tensor("ones_col", [128, 8], F32R, kind="ExternalInput").ap()
    outT_d = nc.dram_tensor("outT", [C, S], F32, kind="ExternalOutput").ap()

    with tile.TileContext(nc) as tc:
        with (
            tc.tile_pool(name="res", bufs=1) as res,
            tc.tile_pool(name="wres", bufs=1) as wres,
            tc.tile_pool(name="xs", bufs=2) as xs,
            tc.tile_pool(name="ex", bufs=6) as exs,
            tc.tile_pool(name="yres", bufs=1) as yres,
            tc.tile_pool(name="rcs", bufs=2) as rcs,
            tc.tile_pool(name="outs", bufs=4) as outs,
            tc.tile_pool(name="sc", bufs=2, space="PSUM") as scp,
            tc.tile_pool(name="ya", bufs=2, space="PSUM") as yap,
            tc.tile_pool(name="mm", bufs=2, space="PSUM") as mmp,
        ):
            # persistent SBUF tiles
            qkT = [res.tile([128, S], BF16, tag=f"qkT{m}", name=f"qkT{m}") for m in range(8)]
            vt = [res.tile([128, HG * 65], F32R, tag=f"vt{m}", name=f"vt{m}") for m in range(S // SK)]
            tri = res.tile([128, 128], F32, tag="tri")
            bqk_t = res.tile([128, 8], F32, tag="bqk")
            bp_t = res.tile([128, 8], F32, tag="bp")
            bv_t = res.tile([1, 512], F32, tag="bv")
            bvb = res.tile([128, 512], F32, tag="bvb")
            yT = [yres.tile([128, S], F32R, tag=f"yT{p}", name=f"yT{p}") for p in range(4)]
            wp_sb = yres.tile([128, 4 * C], F32R, tag="wp")
            wqk_sb = wres.tile([128, KC * 1024], BF16, tag="wqk")
            wv_sb = wres.tile([128, KC * 512], BF16, tag="wv")

            xT_r = xT_d.rearrange("p (k t) -> p k t", t=S)
            # split the first-window loads so chain k=0 starts after 1/4 of
            # the bytes (subtile deps let matmul k wait only on its segment)
            xall0 = xs.tile([128, KC * 512], BF16, tag="xall", name="xall")
            half = KC * 512  # wqk halves in elements
            nc.sync.dma_start(wqk_sb[:, 0:half], wqk_d[:, 0:half])
            nc.sync.dma_start(
                xall0[:, 0:4 * 512].rearrange("p (k t) -> p k t", t=512),
                xT_r[:, 0:4, 0:512],
            )
            nc.sync.dma_start(wqk_sb[:, half:2 * half], wqk_d[:, half:2 * half])
            nc.sync.dma_start(
                xall0[:, 4 * 512:8 * 512].rearrange("p (k t) -> p k t", t=512),
                xT_r[:, 4:8, 0:512],
            )
            nc.sync.dma_start(bqk_t[:], bqk_d[:])
            nc.sync.dma_start(wv_sb[:], wv_d[:])
            nc.sync.dma_start(bv_t[:], bv_d[:])
            nc.gpsimd.partition_broadcast(bvb[:], bv_t[:])
            nc.sync.dma_start(tri[:], tri_d[:])
            nc.sync.dma_start(bp_t[:], bp_d[:])
            for m in range(S // SK):
                nc.sync.dma_start(
                    vt[m][:].rearrange("p (h e) -> p h e", e=65)[:, :, 64:65],
                    onesc_d[:].rearrange("p (h e) -> p h e", e=1),
                )
            nc.sync.dma_start(wp_sb[:], wp_d[:])

            def ab_units(n):
                """qkv-projection work for token chunk n: 12 chain closures."""
                if n == 0:
                    xall = xall0
                else:
                    xall = xs.tile([128, KC * 512], BF16, tag="xall", name="xall")
                    nc.sync.dma_start(
                        xall[:].rearrange("p (k t) -> p k t", t=512),
                        xT_r[:, :, n * 512:(n + 1) * 512],
                    )

                def qk_chain(m):
                    qkp = mmp.tile([128, 512], F32, tag="mm", name="qkp")
                    for k in range(KC):
                        nc.tensor.matmul(
                            qkp[:],
                            wqk_sb[:, k * 1024 + m * 128:k * 1024 + (m + 1) * 128],
                            xall[:, k * 512:(k + 1) * 512],
                            start=(k == 0), stop=(k == KC - 1),
                        )
                    nc.scalar.activation(
                        qkT[m][:, n * 512:(n + 1) * 512], qkp[:],
                        AF.Identity, bias=bqk_t[:, m:m + 1],
                    )

                def v_chain(j):
                    mtok = n * 4 + j
                    vp = mmp.tile([128, 512], F32, tag="mm", name="vp")
                    for k in range(KC):
                        nc.tensor.matmul(
                            vp[:],
                            xall[:, k * 512 + j * 128:k * 512 + (j + 1) * 128],
                            wv_sb[:, k * 512:(k + 1) * 512],
                            start=(k == 0), stop=(k == KC - 1),
                        )
                    nc.vector.tensor_add(
                        vt[mtok][:].rearrange("p (h e) -> p h e", e=65)[:, :, 0:64],
                        vp[:].rearrange("p (h e) -> p h e", e=64),
                        bvb[:].rearrange("p (h e) -> p h e", e=64),
                    )

                units = []
                for m in range(8):
                    units.append(lambda m=m: qk_chain(m))
                for j in range(4):
                    units.append(lambda j=j: v_chain(j))
                return units

            def attn_block(qc, hp):
                """Attention for one head pair at one q chunk."""
                qt = qkT[hp]
                kt = qkT[4 + hp]
                nkc = qc * 4 + 4
                yas = [yap.tile([65, 512], F32, tag="ya", name="ya") for _ in range(2)]
                for kc in range(nkc):
                    d = kc - qc * 4
                    c0 = 128 * d if d > 0 else 0   # first causally-valid column
                    # two-bank tile: head 2*hp in cols 0:512, 2*hp+1 in 512:1024
                    sc = scp.tile([128, 1024], F32, tag="sc", name="sc")
                    for s in range(2):
                        base = 64 * s
                        nc.tensor.matmul(
                            sc[:, s * 512 + c0:(s + 1) * 512],
                            kt[base:base + 64, kc * 128:(kc + 1) * 128],
                            qt[base:base + 64, qc * 512 + c0:(qc + 1) * 512],
                            start=True, stop=True,
                            tile_position=(base, 0),
                        )
                    ex = exs.tile([128, 1024], F32R, tag="ex", name="ex")
                    sc3 = sc[:].rearrange("p (s q) -> p s q", s=2)
                    ex3 = ex[:].rearrange("p (s q) -> p s q", s=2)
                    nc.scalar.activation(
                        ex3[:, :, c0:512], sc3[:, :, c0:512], AF.Exp, scale=0.125
                    )
                    if d >= 0:
                        for s in range(2):
                            nc.vector.tensor_mul(
                                ex[:, s * 512 + 128 * d:s * 512 + 128 * (d + 1)],
                                ex[:, s * 512 + 128 * d:s * 512 + 128 * (d + 1)],
                                tri[:],
                            )
                    for s in range(2):
                        h = 2 * hp + s
                        nc.tensor.matmul(
                            yas[s][:, c0:512],
                            vt[kc][:, h * 65:(h + 1) * 65],
                            ex[:, s * 512 + c0:(s + 1) * 512],
                            start=(kc == 0), stop=(kc == nkc - 1),
                        )
                for s in range(2):
                    base = 64 * s
                    ya = yas[s]
                    # single fast PSUM->SBUF copy releases the psum bank;
                    # normalize proceeds off the critical path
                    ycop = rcs.tile([65, 512], F32, tag="ycop", name="ycop")
                    nc.vector.tensor_copy(ycop[:], ya[:])
                    rcp = rcs.tile([1, 512], F32, tag="rcp", name="rcp", bufs=1)
                    nc.vector.reciprocal(rcp[:], ycop[64:65, :])
                    rbs = rcs.tile([64, 512], F32, tag="rbs", name="rbs", bufs=1)
                    nc.gpsimd.partition_broadcast(rbs[:], rcp[:])
                    nc.vector.tensor_mul(
                        yT[hp][base:base + 64, qc * 512:(qc + 1) * 512],
                        ycop[0:64, :],
                        rbs[:],
                    )

            def proj_unit(qc, mm):
                op = mmp.tile([128, 512], F32, tag="mm", name="op")
                for k in range(4):
                    nc.tensor.matmul(
                        op[:],
                        wp_sb[:, k * 1024 + mm * 128:k * 1024 + (mm + 1) * 128],
                        yT[k][:, qc * 512:(qc + 1) * 512],
                        start=(k == 0), stop=(k == 3),
                    )
                ot = outs.tile([128, 512], F32, tag="ot", name="ot")
                nc.vector.tensor_scalar_add(ot[:], op[:], bp_t[:, mm:mm + 1])
                nc.sync.dma_start(
                    outT_d[mm * 128:(mm + 1) * 128, qc * 512:(qc + 1) * 512],
                    ot[:],
                )

            def weave(a_list, b_list):
                """Cost-weighted interleave: (cost, fn) lists -> emit order."""
                if not a_list:
                    return [f for _, f in b_list]
                if not b_list:
                    return [f for _, f in a_list]
                ta = sum(c for c, _ in a_list)
                tb = sum(c for c, _ in b_list)
                out = []
                ai = iter(a_list)
                acc_a = 0.0
                emitted_a = 0.0
                acc_b = 0.0
                pend = list(a_list)
                pi = 0
                for cb, fb in b_list:
                    acc_b += cb
                    # emit a-units until their cost fraction catches up
                    while pi < len(pend) and emitted_a / ta < acc_b / tb:
                        ca, fa = pend[pi]
                        out.append(fa)
                        emitted_a += ca
                        pi += 1
                    out.append(fb)
                out.extend(f for _, f in pend[pi:])
                return out

            # window n: qkv for chunk n woven with attention for chunk n-1;
            # projection deferred one extra window (qc = n-2) so its PE work
            # lands in the ACT-paced tail instead of the PE-bound middle
            for n in range(NQ + 4):
                a = []
                if n < NQ:
                    a = [(1.0, u) for u in ab_units(n)]
                b = []
                if 1 <= n <= NQ:
                    qc = n - 1
                    for hp in range(HG // 2):
                        b.append((0.5 + 0.5 * qc, lambda qc=qc, hp=hp: attn_block(qc, hp)))
                if n >= 4:
                    qp = n - 4
                    for mm in range(8):
                        b.append((0.4, lambda qp=qp, mm=mm: proj_unit(qp, mm)))
                for unit in weave(a, b):
                    unit()

    nc.compile()
    _BUILT = nc
    return nc


def make_in_maps(x, W_attn, b_attn, W_proj, b_proj):
    import ml_dtypes
    bf16 = ml_dtypes.bfloat16
    x = np.asarray(x, np.float32)
    W_attn = np.asarray(W_attn, np.float32)
    b_attn = np.asarray(b_attn, np.float32)
    W_proj = np.asarray(W_proj, np.float32)
    b_proj = np.asarray(b_proj, np.float32)
    i = np.arange(128)[:, None]
    j = np.arange(128)[None, :]
    tri = (j >= i).astype(np.float32)
    B = x.shape[0]
    # xp[p, k*S + t] = x[b][t, k*128 + p]
    xTs = [np.ascontiguousarray(
        x[b].T.reshape(KC, 128, S).transpose(1, 0, 2).reshape(128, KC * S).astype(bf16)
    ) for b in range(B)]
    in_maps = []
    for b in range(B):
        for g in range(G):
            sl = slice(g * 512, (g + 1) * 512)
            wqk = np.concatenate(
                [W_attn[:, sl], W_attn[:, C + g * 512:C + (g + 1) * 512]], axis=1
            )
            wqk = np.ascontiguousarray(
                wqk.reshape(KC, 128, 1024).transpose(1, 0, 2).reshape(128, KC * 1024)
            ).astype(bf16)
            wv = W_attn[:, 2 * C + g * 512:2 * C + (g + 1) * 512]
            wv = np.ascontiguousarray(
                wv.reshape(KC, 128, 512).transpose(1, 0, 2).reshape(128, KC * 512)
            ).astype(bf16)
            bqk = np.ascontiguousarray(
                np.concatenate([b_attn[sl], b_attn[C + g * 512:C + (g + 1) * 512]])
                .reshape(8, 128).T
            )
            bv = np.ascontiguousarray(b_attn[2 * C + g * 512:2 * C + (g + 1) * 512][None, :])
            wp = np.ascontiguousarray(
                W_proj[g * 512:(g + 1) * 512, :]
                .reshape(4, 128, C).transpose(1, 0, 2).reshape(128, 4 * C)
            )
            bp = np.ascontiguousarray(
                (b_proj if g == 0 else np.zeros_like(b_proj)).reshape(8, 128).T
            )
            in_maps.append({
                "xp": xTs[b], "wqkp": wqk, "wvp": wv, "bqk": bqk, "bv": bv,
                "wpp": wp, "bp": bp, "tri": tri,
                "ones_col": np.ones((128, 8), np.float32),
            })
    return in_maps


def kernel(x, W_attn, b_attn, W_proj, b_proj):
    from concourse.bass_utils import run_bass_kernel_spmd

    nc = _build()
    in_maps = make_in_maps(x, W_attn, b_attn, W_proj, b_proj)
    res = run_bass_kernel_spmd(nc, in_maps, list(range(N_CORES)))
    B = x.shape[0]
    out = np.empty((B, S, C), np.float32)
    for b in range(B):
        out[b] = (res.results[2 * b]["outT"] + res.results[2 * b + 1]["outT"]).T
    return out



# revision 22
# speedup vs baseline: 144.2509x; 144.2509x over previous
"""Causal self-attention (B=4, S=2048, C=1024, 16 heads) on 8 Trainium2 cores.

Sharding: 8 cores = 4 batches x 2 head-groups (8 heads each).
Each core computes, for its (batch b, head-group g):
  qkT = (Wqk_g.T @ x_b.T) + bqk_g          [1024, 2048]  (q rows then k rows)
  v   = (x_b @ Wv_g) + bv_g                [2048, 520]   (65-strided heads, ones col)
  per head h, q-chunk qc (512 wide), k-chunk kc (128 wide, causal band only):
    scoresT = kT_chunk.T @ qT_chunk        [128, <=512]  (K = head dim 64)
    expT    = exp(0.125 * scoresT); triangular mask on the diagonal subtile
    yaccT  += [v_chunk | ones].T @ expT    [65, <=512]   (row 64 = softmax denom)
  yT = yaccT[0:64] * broadcast(1/yaccT[64])               (broadcast via K=1 matmul)
  outT_partial = Wproj_rows_g.T @ yT + bproj (g==0 only)  [1024, 2048]
Host sums the two head-group partials per batch and transposes.

Matmuls run as float32r (TF32, full PE rate at free-dim >=256). Score matmuls
for even/odd head pairs are row-packed into PE array halves via tile_position.
Diagonal-band blocks are column-trimmed to the causal region.
"""
import sys

sys.path.insert(0, "/opt/trn_rl_repo")

import numpy as np

S = 2048
C = 1024
NH = 16
NE = 64
G = 2                 # head groups (tensor-parallel factor)
HG = NH // G          # heads per core = 8
SQ = 512              # q chunk
SK = 128              # k chunk
KC = C // 128         # 8 contraction chunks for qkv projections
NQ = S // SQ
N_CORES = 8

_BUILT = None


def _build():
    global _BUILT
    if _BUILT is not None:
        return _BUILT

    import concourse.bacc as bacc
    import concourse.mybir as mybir
    from concourse import tile

    dt = mybir.dt
    F32 = dt.float32
    F32R = dt.float32r
    AF = mybir.ActivationFunctionType
    BF16 = dt.bfloat16

    nc = bacc.Bacc("TRN2", target_bir_lowering=False, debug=False, num_devices=N_CORES)

    xT_d = nc.dram_tensor("xp", [128, KC * S], BF16, kind="ExternalInput").ap()
    wqk_d = nc.dram_tensor("wqkp", [128, KC * 1024], BF16, kind="ExternalInput").ap()
    wv_d = nc.dram_tensor("wvp", [128, KC * 512], BF16, kind="ExternalInput").ap()
    bqk_d = nc.dram_tensor("bqk", [128, 8], F32, kind="ExternalInput").ap()
    bv_d = nc.dram_tensor("bv", [1, 512], F32, kind="ExternalInput").ap()
    wp_d = nc.dram_tensor("wpp", [128, 4 * C], BF16, kind="ExternalInput").ap()
    bp_d = nc.dram_tensor("bp", [128, 8], F32, kind="ExternalInput").ap()
    tri_d = nc.dram_tensor("tri", [128, 128], BF16, kind="ExternalInput").ap()
    onesc_d = nc.dram_tensor("ones_col", [128, (S // SK) * HG], BF16, kind="ExternalInput").ap()
    outT_d = nc.dram_tensor("outT", [C, S], BF16, kind="ExternalOutput").ap()

    with tile.TileContext(nc) as tc:
        with (
            tc.tile_pool(name="res", bufs=1) as res,
            tc.tile_pool(name="wres", bufs=1) as wres,
            tc.tile_pool(name="xs", bufs=2) as xs,
            tc.tile_pool(name="ex", bufs=6) as exs,
            tc.tile_pool(name="yres", bufs=1) as yres,
            tc.tile_pool(name="rcs", bufs=2) as rcs,
            tc.tile_pool(name="outs", bufs=4) as outs,
            tc.tile_pool(name="sc", bufs=2, space="PSUM") as scp,
            tc.tile_pool(name="ya", bufs=2, space="PSUM") as yap,
            tc.tile_pool(name="mm", bufs=2, space="PSUM") as mmp,
        ):
            # persistent SBUF tiles
            qkT = [res.tile([128, S], BF16, tag=f"qkT{m}", name=f"qkT{m}") for m in range(8)]
            vt = [res.tile([128, HG * 65], BF16, tag=f"vt{m}", name=f"vt{m}") for m in range(S // SK)]
            tri = res.tile([128, 128], BF16, tag="tri")
            bqk_t = res.tile([128, 8], F32, tag="bqk")
            bp_t = res.tile([128, 8], F32, tag="bp")
            bv_t = res.tile([1, 512], F32, tag="bv")
            bvb = res.tile([128, 512], F32, tag="bvb")
            yT = [yres.tile([128, S], BF16, tag=f"yT{p}", name=f"yT{p}") for p in range(4)]
            wp_sb = yres.tile([128, 4 * C], BF16, tag="wp")
            wqk_sb = wres.tile([128, KC * 1024], BF16, tag="wqk")
            wv_sb = wres.tile([128, KC * 512], BF16, tag="wv")

            xT_r = xT_d.rearrange("p (k t) -> p k t", t=S)
            # split the first-window loads so chain k=0 starts after 1/4 of
            # the bytes (subtile deps let matmul k wait only on its segment)
            xall0 = xs.tile([128, KC * 512], BF16, tag="xall", name="xall")
            half = KC * 512  # wqk halves in elements
            nc.sync.dma_start(wqk_sb[:, 0:half], wqk_d[:, 0:half])
            nc.sync.dma_start(
                xall0[:, 0:4 * 512].rearrange("p (k t) -> p k t", t=512),
                xT_r[:, 0:4, 0:512],
            )
            nc.sync.dma_start(wqk_sb[:, half:2 * half], wqk_d[:, half:2 * half])
            nc.sync.dma_start(
                xall0[:, 4 * 512:8 * 512].rearrange("p (k t) -> p k t", t=512),
                xT_r[:, 4:8, 0:512],
            )
            nc.sync.dma_start(bqk_t[:], bqk_d[:])
            nc.sync.dma_start(wv_sb[:], wv_d[:])
            nc.sync.dma_start(bv_t[:], bv_d[:])
            nc.gpsimd.partition_broadcast(bvb[:], bv_t[:])
            nc.sync.dma_start(tri[:], tri_d[:])
            nc.sync.dma_start(bp_t[:], bp_d[:])
            for m in range(S // SK):
                nc.sync.dma_start(
                    vt[m][:].rearrange("p (h e) -> p h e", e=65)[:, :, 64:65],
                    onesc_d[:, m * 8:(m + 1) * 8].rearrange("p (h e) -> p h e", e=1),
                )
            # wp on the ACT hwdge queue: keeps the SP queue free for x loads
            nc.scalar.dma_start(wp_sb[:], wp_d[:])

            def ab_units(n):
                """qkv-projection work for token chunk n: 12 chain closures."""
                if n == 0:
                    xall = xall0
                else:
                    xall = xs.tile([128, KC * 512], BF16, tag="xall", name="xall")
                    nc.sync.dma_start(
                        xall[:].rearrange("p (k t) -> p k t", t=512),
                        xT_r[:, :, n * 512:(n + 1) * 512],
                    )

                def qk_chain(m):
                    qkp = mmp.tile([128, 512], F32, tag="mm", name="qkp")
                    for k in range(KC):
                        nc.tensor.matmul(
                            qkp[:],
                            wqk_sb[:, k * 1024 + m * 128:k * 1024 + (m + 1) * 128],
                            xall[:, k * 512:(k + 1) * 512],
                            start=(k == 0), stop=(k == KC - 1),
                        )
                    nc.vector.tensor_scalar_add(
                        qkT[m][:, n * 512:(n + 1) * 512], qkp[:],
                        bqk_t[:, m:m + 1],
                    )

                def v_chain(j):
                    mtok = n * 4 + j
                    vp = mmp.tile([128, 512], F32, tag="mm", name="vp")
                    for k in range(KC):
                        nc.tensor.matmul(
                            vp[:],
                            xall[:, k * 512 + j * 128:k * 512 + (j + 1) * 128],
                            wv_sb[:, k * 512:(k + 1) * 512],
                            start=(k == 0), stop=(k == KC - 1),
                        )
                    nc.vector.tensor_add(
                        vt[mtok][:].rearrange("p (h e) -> p h e", e=65)[:, :, 0:64],
                        vp[:].rearrange("p (h e) -> p h e", e=64),
                        bvb[:].rearrange("p (h e) -> p h e", e=64),
                    )

                units = []
                for m in range(8):
                    units.append(lambda m=m: qk_chain(m))
                for j in range(4):
                    units.append(lambda j=j: v_chain(j))
                return units

            def attn_block(qc, hp):
                """Attention for one head pair at one q chunk."""
                qt = qkT[hp]
                kt = qkT[4 + hp]
                nkc = qc * 4 + 4
                yas = [yap.tile([65, 512], F32, tag="ya", name="ya") for _ in range(2)]
                for kc in range(nkc):
                    d = kc - qc * 4
                    c0 = 128 * d if d > 0 else 0   # first causally-valid column
                    # two-bank tile: head 2*hp in cols 0:512, 2*hp+1 in 512:1024
                    sc = scp.tile([128, 1024], F32, tag="sc", name="sc")
                    for s in range(2):
                        base = 64 * s
                        nc.tensor.matmul(
                            sc[:, s * 512 + c0:(s + 1) * 512],
                            kt[base:base + 64, kc * 128:(kc + 1) * 128],
                            qt[base:base + 64, qc * 512 + c0:(qc + 1) * 512],
                            start=True, stop=True,
                            tile_position=(base, 0),
                        )
                    ex = exs.tile([128, 1024], BF16, tag="ex", name="ex")
                    sc3 = sc[:].rearrange("p (s q) -> p s q", s=2)
                    ex3 = ex[:].rearrange("p (s q) -> p s q", s=2)
                    nc.scalar.activation(
                        ex3[:, :, c0:512], sc3[:, :, c0:512], AF.Exp, scale=0.125
                    )
                    if d >= 0:
                        for s in range(2):
                            nc.vector.tensor_mul(
                                ex[:, s * 512 + 128 * d:s * 512 + 128 * (d + 1)],
                                ex[:, s * 512 + 128 * d:s * 512 + 128 * (d + 1)],
                                tri[:],
                            )
                    for s in range(2):
                        h = 2 * hp + s
                        nc.tensor.matmul(
                            yas[s][:, c0:512],
                            vt[kc][:, h * 65:(h + 1) * 65],
                            ex[:, s * 512 + c0:(s + 1) * 512],
                            start=(kc == 0), stop=(kc == nkc - 1),
                        )
                for s in range(2):
                    base = 64 * s
                    ya = yas[s]
                    # single fast PSUM->SBUF copy releases the psum bank;
                    # normalize proceeds off the critical path
                    ycop = rcs.tile([65, 512], F32, tag="ycop", name="ycop")
                    nc.vector.tensor_copy(ycop[:], ya[:])
                    den = rcs.tile([1, 512], F32, tag="den", name="den", bufs=1)
                    nc.vector.tensor_copy(den[:], ya[64:65, :])
                    rcp = rcs.tile([1, 512], F32, tag="rcp", name="rcp", bufs=1)
                    nc.vector.reciprocal_approx_fast(rcp[:], den[:])
                    rbs = rcs.tile([64, 512], F32, tag="rbs", name="rbs", bufs=1)
                    nc.gpsimd.partition_broadcast(rbs[:], rcp[:])
                    nc.vector.tensor_mul(
                        yT[hp][base:base + 64, qc * 512:(qc + 1) * 512],
                        ycop[0:64, :],
                        rbs[:],
                    )

            def proj_unit(qc, mm):
                op = mmp.tile([128, 512], F32, tag="mm", name="op")
                for k in range(4):
                    nc.tensor.matmul(
                        op[:],
                        wp_sb[:, k * 1024 + mm * 128:k * 1024 + (mm + 1) * 128],
                        yT[k][:, qc * 512:(qc + 1) * 512],
                        start=(k == 0), stop=(k == 3),
                    )
                ot = outs.tile([128, 512], BF16, tag="ot", name="ot")
                nc.vector.tensor_scalar_add(ot[:], op[:], bp_t[:, mm:mm + 1])
                nc.sync.dma_start(
                    outT_d[mm * 128:(mm + 1) * 128, qc * 512:(qc + 1) * 512],
                    ot[:],
                )

            def weave(a_list, b_list):
                """Cost-weighted interleave: (cost, fn) lists -> emit order."""
                if not a_list:
                    return [f for _, f in b_list]
                if not b_list:
                    return [f for _, f in a_list]
                ta = sum(c for c, _ in a_list)
                tb = sum(c for c, _ in b_list)
                out = []
                ai = iter(a_list)
                acc_a = 0.0
                emitted_a = 0.0
                acc_b = 0.0
                pend = list(a_list)
                pi = 0
                for cb, fb in b_list:
                    acc_b += cb
                    # emit a-units until their cost fraction catches up
                    while pi < len(pend) and emitted_a / ta < acc_b / tb:
                        ca, fa = pend[pi]
                        out.append(fa)
                        emitted_a += ca
                        pi += 1
                    out.append(fb)
                out.extend(f for _, f in pend[pi:])
                return out

            # window n: qkv for chunk n woven with attention for chunk n-1;
            # projection deferred one extra window (qc = n-2) so its PE work
            # lands in the ACT-paced tail instead of the PE-bound middle
            for n in range(NQ + 4):
                a = []
                if n < NQ:
                    a = [(1.0, u) for u in ab_units(n)]
                b = []
                if 1 <= n <= NQ:
                    qc = n - 1
                    for hp in range(HG // 2):
                        b.append((0.5 + 0.5 * qc, lambda qc=qc, hp=hp: attn_block(qc, hp)))
                if n >= 4:
                    qp = n - 4
                    for mm in range(8):
                        b.append((0.4, lambda qp=qp, mm=mm: proj_unit(qp, mm)))
                for unit in weave(a, b):
                    unit()

    nc.compile()
    _BUILT = nc
    return nc


def make_in_maps(x, W_attn, b_attn, W_proj, b_proj):
    import ml_dtypes
    bf16 = ml_dtypes.bfloat16
    x = np.asarray(x, np.float32)
    W_attn = np.asarray(W_attn, np.float32)
    b_attn = np.asarray(b_attn, np.float32)
    W_proj = np.asarray(W_proj, np.float32)
    b_proj = np.asarray(b_proj, np.float32)
    i = np.arange(128)[:, None]
    j = np.arange(128)[None, :]
    tri = (j >= i).astype(bf16)
    B = x.shape[0]
    # xp[p, k*S + t] = x[b][t, k*128 + p]
    xTs = [np.ascontiguousarray(
        x[b].T.reshape(KC, 128, S).transpose(1, 0, 2).reshape(128, KC * S).astype(bf16)
    ) for b in range(B)]
    in_maps = []
    for b in range(B):
        for g in range(G):
            sl = slice(g * 512, (g + 1) * 512)
            wqk = np.concatenate(
                [W_attn[:, sl], W_attn[:, C + g * 512:C + (g + 1) * 512]], axis=1
            )
            wqk = np.ascontiguousarray(
                wqk.reshape(KC, 128, 1024).transpose(1, 0, 2).reshape(128, KC * 1024)
            ).astype(bf16)
            wv = W_attn[:, 2 * C + g * 512:2 * C + (g + 1) * 512]
            wv = np.ascontiguousarray(
                wv.reshape(KC, 128, 512).transpose(1, 0, 2).reshape(128, KC * 512)
            ).astype(bf16)
            bqk = np.ascontiguousarray(
                np.concatenate([b_attn[sl], b_attn[C + g * 512:C + (g + 1) * 512]])
                .reshape(8, 128).T
            )
            bv = np.ascontiguousarray(b_attn[2 * C + g * 512:2 * C + (g + 1) * 512][None, :])
            wp = np.ascontiguousarray(
                W_proj[g * 512:(g + 1) * 512, :]
                .reshape(4, 128, C).transpose(1, 0, 2).reshape(128, 4 * C)
            ).astype(bf16)
            bp = np.ascontiguousarray(
                (b_proj if g == 0 else np.zeros_like(b_proj)).reshape(8, 128).T
            )
            in_maps.append({
                "xp": xTs[b], "wqkp": wqk, "wvp": wv, "bqk": bqk, "bv": bv,
                "wpp": wp, "bp": bp, "tri": tri,
                "ones_col": np.ones((128, 128), bf16),
            })
    return in_maps


def kernel(x, W_attn, b_attn, W_proj, b_proj):
    from concourse.bass_utils import run_bass_kernel_spmd

    nc = _build()
    in_maps = make_in_maps(x, W_attn, b_attn, W_proj, b_proj)
    res = run_bass_kernel_spmd(nc, in_maps, list(range(N_CORES)))
    B = x.shape[0]
    out = np.empty((B, S, C), np.float32)
    for b in range(B):
        out[b] = (np.asarray(res.results[2 * b]["outT"], np.float32)
                  + np.asarray(res.results[2 * b + 1]["outT"], np.float32)).T
    return out



# revision 23
# speedup vs baseline: 149.2750x; 1.0348x over previous
"""Causal self-attention (B=4, S=2048, C=1024, 16 heads) on 8 Trainium2 cores.

Sharding: 8 cores = 4 batches x 2 head-groups (8 heads each).
Each core computes, for its (batch b, head-group g):
  qkT = (Wqk_g.T @ x_b.T) + bqk_g          [1024, 2048]  (q rows then k rows)
  v   = (x_b @ Wv_g) + bv_g                [2048, 520]   (65-strided heads, ones col)
  per head h, q-chunk qc (512 wide), k-chunk kc (128 wide, causal band only):
    scoresT = kT_chunk.T @ qT_chunk        [128, <=512]  (K = head dim 64)
    expT    = exp(0.125 * scoresT); triangular mask on the diagonal subtile
    yaccT  += [v_chunk | ones].T @ expT    [65, <=512]   (row 64 = softmax denom)
  yT = yaccT[0:64] * broadcast(1/yaccT[64])               (broadcast via K=1 matmul)
  outT_partial = Wproj_rows_g.T @ yT + bproj (g==0 only)  [1024, 2048]
Host sums the two head-group partials per batch and transposes.

Matmuls run as float32r (TF32, full PE rate at free-dim >=256). Score matmuls
for even/odd head pairs are row-packed into PE array halves via tile_position.
Diagonal-band blocks are column-trimmed to the causal region.
"""
import sys

sys.path.insert(0, "/opt/trn_rl_repo")

import numpy as np

S = 2048
C = 1024
NH = 16
NE = 64
G = 2                 # head groups (tensor-parallel factor)
HG = NH // G          # heads per core = 8
SQ = 512              # q chunk
SK = 128              # k chunk
KC = C // 128         # 8 contraction chunks for qkv projections
NQ = S // SQ
N_CORES = 8

_BUILT = None


def _build():
    global _BUILT
    if _BUILT is not None:
        return _BUILT

    import concourse.bacc as bacc
    import concourse.mybir as mybir
    from concourse import tile

    dt = mybir.dt
    F32 = dt.float32
    F32R = dt.float32r
    AF = mybir.ActivationFunctionType
    BF16 = dt.bfloat16

    nc = bacc.Bacc("TRN2", target_bir_lowering=False, debug=False, num_devices=N_CORES)

    xT_d = nc.dram_tensor("xp", [128, KC * S], BF16, kind="ExternalInput").ap()
    wqk_d = nc.dram_tensor("wqkp", [128, KC * 1024], BF16, kind="ExternalInput").ap()
    wv_d = nc.dram_tensor("wvp", [128, KC * 512], BF16, kind="ExternalInput").ap()
    bqk_d = nc.dram_tensor("bqk", [128, 8], F32, kind="ExternalInput").ap()
    bv_d = nc.dram_tensor("bv", [1, 512], F32, kind="ExternalInput").ap()
    wp_d = nc.dram_tensor("wpp", [128, 4 * C], BF16, kind="ExternalInput").ap()
    bp_d = nc.dram_tensor("bp", [128, 8], F32, kind="ExternalInput").ap()
    tri_d = nc.dram_tensor("tri", [128, 128], BF16, kind="ExternalInput").ap()
    onesc_d = nc.dram_tensor("ones_col", [128, (S // SK) * HG], BF16, kind="ExternalInput").ap()
    outT_d = nc.dram_tensor("outT", [C, S], BF16, kind="ExternalOutput").ap()

    with tile.TileContext(nc) as tc:
        with (
            tc.tile_pool(name="res", bufs=1) as res,
            tc.tile_pool(name="wres", bufs=1) as wres,
            tc.tile_pool(name="xs", bufs=2) as xs,
            tc.tile_pool(name="ex", bufs=6) as exs,
            tc.tile_pool(name="yres", bufs=1) as yres,
            tc.tile_pool(name="rcs", bufs=2) as rcs,
            tc.tile_pool(name="outs", bufs=4) as outs,
            tc.tile_pool(name="sc", bufs=2, space="PSUM") as scp,
            tc.tile_pool(name="ya", bufs=2, space="PSUM") as yap,
            tc.tile_pool(name="mm", bufs=2, space="PSUM") as mmp,
        ):
            # persistent SBUF tiles
            qkT = [res.tile([128, S], BF16, tag=f"qkT{m}", name=f"qkT{m}") for m in range(8)]
            vt = [res.tile([128, HG * 65], BF16, tag=f"vt{m}", name=f"vt{m}") for m in range(S // SK)]
            tri = res.tile([128, 128], BF16, tag="tri")
            bqk_t = res.tile([128, 8], F32, tag="bqk")
            bp_t = res.tile([128, 8], F32, tag="bp")
            bv_t = res.tile([1, 512], F32, tag="bv")
            bvb = res.tile([128, 512], F32, tag="bvb")
            yT = [yres.tile([128, S], BF16, tag=f"yT{p}", name=f"yT{p}") for p in range(4)]
            wp_sb = yres.tile([128, 4 * C], BF16, tag="wp")
            wqk_sb = wres.tile([128, KC * 1024], BF16, tag="wqk")
            wv_sb = wres.tile([128, KC * 512], BF16, tag="wv")

            xT_r = xT_d.rearrange("p (k t) -> p k t", t=S)
            # split the first-window loads so chain k=0 starts after 1/4 of
            # the bytes (subtile deps let matmul k wait only on its segment)
            xall0 = xs.tile([128, KC * 512], BF16, tag="xall", name="xall")
            # per-k interleaved loads: chain (m, k) can start as soon as its
            # own 384KB (wqk k-chunk + x k-chunk) has landed
            for k in range(KC):
                nc.sync.dma_start(
                    wqk_sb[:, k * 1024:(k + 1) * 1024],
                    wqk_d[:, k * 1024:(k + 1) * 1024],
                )
                nc.sync.dma_start(xall0[:, k * 512:(k + 1) * 512], xT_r[:, k, 0:512])
            nc.sync.dma_start(bqk_t[:], bqk_d[:])
            nc.sync.dma_start(wv_sb[:], wv_d[:])
            nc.sync.dma_start(bv_t[:], bv_d[:])
            nc.gpsimd.partition_broadcast(bvb[:], bv_t[:])
            nc.sync.dma_start(tri[:], tri_d[:])
            nc.sync.dma_start(bp_t[:], bp_d[:])
            for m in range(S // SK):
                nc.sync.dma_start(
                    vt[m][:].rearrange("p (h e) -> p h e", e=65)[:, :, 64:65],
                    onesc_d[:, m * 8:(m + 1) * 8].rearrange("p (h e) -> p h e", e=1),
                )

            def ab_units(n):
                """qkv-projection work for token chunk n: 12 chain closures."""
                if n == 0:
                    xall = xall0
                else:
                    xall = xs.tile([128, KC * 512], BF16, tag="xall", name="xall")
                    nc.sync.dma_start(
                        xall[:].rearrange("p (k t) -> p k t", t=512),
                        xT_r[:, :, n * 512:(n + 1) * 512],
                    )

                def qk_chain(m):
                    qkp = mmp.tile([128, 512], F32, tag="mm", name="qkp")
                    for k in range(KC):
                        nc.tensor.matmul(
                            qkp[:],
                            wqk_sb[:, k * 1024 + m * 128:k * 1024 + (m + 1) * 128],
                            xall[:, k * 512:(k + 1) * 512],
                            start=(k == 0), stop=(k == KC - 1),
                        )
                    nc.vector.tensor_scalar_add(
                        qkT[m][:, n * 512:(n + 1) * 512], qkp[:],
                        bqk_t[:, m:m + 1],
                    )

                def v_chain(j):
                    mtok = n * 4 + j
                    vp = mmp.tile([128, 512], F32, tag="mm", name="vp")
                    for k in range(KC):
                        nc.tensor.matmul(
                            vp[:],
                            xall[:, k * 512 + j * 128:k * 512 + (j + 1) * 128],
                            wv_sb[:, k * 512:(k + 1) * 512],
                            start=(k == 0), stop=(k == KC - 1),
                        )
                    nc.vector.tensor_add(
                        vt[mtok][:].rearrange("p (h e) -> p h e", e=65)[:, :, 0:64],
                        vp[:].rearrange("p (h e) -> p h e", e=64),
                        bvb[:].rearrange("p (h e) -> p h e", e=64),
                    )

                units = []
                for m in range(8):
                    units.append(lambda m=m: qk_chain(m))
                for j in range(4):
                    units.append(lambda j=j: v_chain(j))
                return units

            def attn_block(qc, hp):
                """Attention for one head pair at one q chunk."""
                qt = qkT[hp]
                kt = qkT[4 + hp]
                nkc = qc * 4 + 4
                yas = [yap.tile([65, 512], F32, tag="ya", name="ya") for _ in range(2)]
                for kc in range(nkc):
                    d = kc - qc * 4
                    c0 = 128 * d if d > 0 else 0   # first causally-valid column
                    # two-bank tile: head 2*hp in cols 0:512, 2*hp+1 in 512:1024
                    sc = scp.tile([128, 1024], F32, tag="sc", name="sc")
                    for s in range(2):
                        base = 64 * s
                        nc.tensor.matmul(
                            sc[:, s * 512 + c0:(s + 1) * 512],
                            kt[base:base + 64, kc * 128:(kc + 1) * 128],
                            qt[base:base + 64, qc * 512 + c0:(qc + 1) * 512],
                            start=True, stop=True,
                            tile_position=(base, 0),
                        )
                    ex = exs.tile([128, 1024], BF16, tag="ex", name="ex")
                    sc3 = sc[:].rearrange("p (s q) -> p s q", s=2)
                    ex3 = ex[:].rearrange("p (s q) -> p s q", s=2)
                    nc.scalar.activation(
                        ex3[:, :, c0:512], sc3[:, :, c0:512], AF.Exp, scale=0.125
                    )
                    if d >= 0:
                        for s in range(2):
                            nc.vector.tensor_mul(
                                ex[:, s * 512 + 128 * d:s * 512 + 128 * (d + 1)],
                                ex[:, s * 512 + 128 * d:s * 512 + 128 * (d + 1)],
                                tri[:],
                            )
                    for s in range(2):
                        h = 2 * hp + s
                        nc.tensor.matmul(
                            yas[s][:, c0:512],
                            vt[kc][:, h * 65:(h + 1) * 65],
                            ex[:, s * 512 + c0:(s + 1) * 512],
                            start=(kc == 0), stop=(kc == nkc - 1),
                        )
                for s in range(2):
                    base = 64 * s
                    ya = yas[s]
                    # single fast PSUM->SBUF copy releases the psum bank;
                    # normalize proceeds off the critical path
                    den = rcs.tile([1, 512], F32, tag="den", name="den", bufs=1)
                    nc.scalar.copy(den[:], ya[64:65, :])
                    rcp = rcs.tile([1, 512], F32, tag="rcp", name="rcp", bufs=1)
                    nc.vector.reciprocal_approx_fast(rcp[:], den[:])
                    rbs = rcs.tile([64, 512], F32, tag="rbs", name="rbs", bufs=1)
                    nc.gpsimd.partition_broadcast(rbs[:], rcp[:])
                    ycop = rcs.tile([64, 512], F32, tag="ycop", name="ycop")
                    nc.vector.tensor_copy(ycop[:], ya[0:64, :])
                    nc.vector.tensor_mul(
                        yT[hp][base:base + 64, qc * 512:(qc + 1) * 512],
                        ycop[:],
                        rbs[:],
                    )

            def proj_unit(qc, mm):
                op = mmp.tile([128, 512], F32, tag="mm", name="op")
                for k in range(4):
                    nc.tensor.matmul(
                        op[:],
                        wp_sb[:, k * 1024 + mm * 128:k * 1024 + (mm + 1) * 128],
                        yT[k][:, qc * 512:(qc + 1) * 512],
                        start=(k == 0), stop=(k == 3),
                    )
                ot = outs.tile([128, 512], BF16, tag="ot", name="ot")
                nc.vector.tensor_scalar_add(ot[:], op[:], bp_t[:, mm:mm + 1])
                nc.sync.dma_start(
                    outT_d[mm * 128:(mm + 1) * 128, qc * 512:(qc + 1) * 512],
                    ot[:],
                )

            def weave(a_list, b_list):
                """Cost-weighted interleave: (cost, fn) lists -> emit order."""
                if not a_list:
                    return [f for _, f in b_list]
                if not b_list:
                    return [f for _, f in a_list]
                ta = sum(c for c, _ in a_list)
                tb = sum(c for c, _ in b_list)
                out = []
                ai = iter(a_list)
                acc_a = 0.0
                emitted_a = 0.0
                acc_b = 0.0
                pend = list(a_list)
                pi = 0
                for cb, fb in b_list:
                    acc_b += cb
                    # emit a-units until their cost fraction catches up
                    while pi < len(pend) and emitted_a / ta < acc_b / tb:
                        ca, fa = pend[pi]
                        out.append(fa)
                        emitted_a += ca
                        pi += 1
                    out.append(fb)
                out.extend(f for _, f in pend[pi:])
                return out

            # window n: qkv for chunk n woven with attention for chunk n-1;
            # projection deferred one extra window (qc = n-2) so its PE work
            # lands in the ACT-paced tail instead of the PE-bound middle
            for n in range(NQ + 4):
                if n == 1:
                    # wp on the ACT hwdge queue, deferred past the startup
                    # x/wqk loads it would otherwise compete with
                    nc.scalar.dma_start(wp_sb[:], wp_d[:])
                a = []
                if n < NQ:
                    a = [(1.0, u) for u in ab_units(n)]
                b = []
                if 1 <= n <= NQ:
                    qc = n - 1
                    for hp in range(HG // 2):
                        b.append((0.5 + 0.5 * qc, lambda qc=qc, hp=hp: attn_block(qc, hp)))
                if n >= 4:
                    qp = n - 4
                    for mm in range(8):
                        b.append((0.4, lambda qp=qp, mm=mm: proj_unit(qp, mm)))
                for unit in weave(a, b):
                    unit()

    nc.compile()
    _BUILT = nc
    return nc


def make_in_maps(x, W_attn, b_attn, W_proj, b_proj):
    import ml_dtypes
    bf16 = ml_dtypes.bfloat16
    x = np.asarray(x, np.float32)
    W_attn = np.asarray(W_attn, np.float32)
    b_attn = np.asarray(b_attn, np.float32)
    W_proj = np.asarray(W_proj, np.float32)
    b_proj = np.asarray(b_proj, np.float32)
    i = np.arange(128)[:, None]
    j = np.arange(128)[None, :]
    tri = (j >= i).astype(bf16)
    B = x.shape[0]
    # xp[p, k*S + t] = x[b][t, k*128 + p]
    xTs = [np.ascontiguousarray(
        x[b].T.reshape(KC, 128, S).transpose(1, 0, 2).reshape(128, KC * S).astype(bf16)
    ) for b in range(B)]
    in_maps = []
    for b in range(B):
        for g in range(G):
            sl = slice(g * 512, (g + 1) * 512)
            wqk = np.concatenate(
                [W_attn[:, sl], W_attn[:, C + g * 512:C + (g + 1) * 512]], axis=1
            )
            wqk = np.ascontiguousarray(
                wqk.reshape(KC, 128, 1024).transpose(1, 0, 2).reshape(128, KC * 1024)
            ).astype(bf16)
            wv = W_attn[:, 2 * C + g * 512:2 * C + (g + 1) * 512]
            wv = np.ascontiguousarray(
                wv.reshape(KC, 128, 512).transpose(1, 0, 2).reshape(128, KC * 512)
            ).astype(bf16)
            bqk = np.ascontiguousarray(
                np.concatenate([b_attn[sl], b_attn[C + g * 512:C + (g + 1) * 512]])
                .reshape(8, 128).T
            )
            bv = np.ascontiguousarray(b_attn[2 * C + g * 512:2 * C + (g + 1) * 512][None, :])
            wp = np.ascontiguousarray(
                W_proj[g * 512:(g + 1) * 512, :]
                .reshape(4, 128, C).transpose(1, 0, 2).reshape(128, 4 * C)
            ).astype(bf16)
            bp = np.ascontiguousarray(
                (b_proj if g == 0 else np.zeros_like(b_proj)).reshape(8, 128).T
            )
            in_maps.append({
                "xp": xTs[b], "wqkp": wqk, "wvp": wv, "bqk": bqk, "bv": bv,
                "wpp": wp, "bp": bp, "tri": tri,
                "ones_col": np.ones((128, 128), bf16),
            })
    return in_maps


def kernel(x, W_attn, b_attn, W_proj, b_proj):
    from concourse.bass_utils import run_bass_kernel_spmd

    nc = _build()
    in_maps = make_in_maps(x, W_attn, b_attn, W_proj, b_proj)
    res = run_bass_kernel_spmd(nc, in_maps, list(range(N_CORES)))
    B = x.shape[0]
    out = np.empty((B, S, C), np.float32)
    for b in range(B):
        out[b] = (np.asarray(res.results[2 * b]["outT"], np.float32)
                  + np.asarray(res.results[2 * b + 1]["outT"], np.float32)).T
    return out

